# revision 1
# baseline (speedup 1.0000x reference)
"""Multi-head attention (B=4, T=2048, D=1024, H=16) on 8 NeuronCores.

Sharding: core c -> batch c//2, head-group c%2 (8 heads = 512 head-dims).
Each core computes its batch's q/k/v projections for its heads, attention,
and a partial output projection (w_o column-slice). Host sums the two
partials per batch and adds b_o. No on-device collectives.

Layouts (per core, all host-side prepped):
  qbT/kbT/vbT : (1024, 2048)  activations transposed
  wqT/wkT/wvT : (1024, 512)   weight slices transposed (dm, hd_local)
  woTs        : (512, 1024)   w_o[:, group_cols].T
  bq/bk       : (128, 4)      bias per m-tile (partition-major)
  bv          : (64, 8)       bias per head (partition-major)
  out         : (2048, 1024)  partial output

On-chip per core:
  qT, kT (hd 512 on 4 partition-tiles x t 2048)  - head pair per tile
  vho    (t2 on partitions x 16 t2-tiles x 8*(64+1)) - v columns + ones col
  scores^T via row-packed K=64 matmul pairs -> PSUM (128 x 1024)
  exp on ScalarE (scale=1/8 fused), F=1024
  att@v: stationary [v_h | 1] (M=65) -> unnormalized out^T + denominators
  normalize: DVE reciprocal -> gpsimd partition_broadcast -> DVE mul + b_v
  o-proj: attnT (128x128) stationary x woTs -> PSUM accum over pairs -> DMA
"""

import numpy as np

import concourse.bacc as bacc
import concourse.mybir as mybir
import concourse.tile as tile
from concourse.bass_utils import run_bass_kernel_spmd

F32 = mybir.dt.float32
F32R = mybir.dt.float32r
EXP = mybir.ActivationFunctionType.Exp

P = 128
DK = 64


def build_nc(T1=2048, T2=2048, DM=1024, HDL=512, num_devices=8, debug_outs=False):
    n_t1q = T1 // 512      # t1 windows (512 wide)
    n_pairs = HDL // P     # head-pairs (= partition tiles of local head dims)
    n_t2t = T2 // P        # t2 tiles
    n_kt = DM // P         # contraction tiles over d_model
    n_no = DM // 512       # o-proj n blocks

    nc = bacc.Bacc("TRN2", target_bir_lowering=False, debug=False,
                   num_devices=num_devices)

    qbT = nc.dram_tensor("qbT", [DM, T1], F32R, kind="ExternalInput")
    kbT = nc.dram_tensor("kbT", [DM, T2], F32R, kind="ExternalInput")
    vbT = nc.dram_tensor("vbT", [DM, T2], F32R, kind="ExternalInput")
    wqT = nc.dram_tensor("wqT", [DM, HDL], F32R, kind="ExternalInput")
    wkT = nc.dram_tensor("wkT", [DM, HDL], F32R, kind="ExternalInput")
    wvT = nc.dram_tensor("wvT", [DM, HDL], F32R, kind="ExternalInput")
    woTs = nc.dram_tensor("woTs", [HDL, DM], F32R, kind="ExternalInput")
    bq = nc.dram_tensor("bq", [P, n_pairs], F32, kind="ExternalInput")
    bk = nc.dram_tensor("bk", [P, n_pairs], F32, kind="ExternalInput")
    bv = nc.dram_tensor("bv", [DK, 2 * n_pairs], F32, kind="ExternalInput")
    ones = nc.dram_tensor("ones", [P, n_t2t * 2 * n_pairs + DK], F32R,
                          kind="ExternalInput")
    out = nc.dram_tensor("out", [T1, DM], F32, kind="ExternalOutput")
    if debug_outs:
        d_qT = nc.dram_tensor("d_qT", [P, (HDL // P) * T1], F32R,
                              kind="ExternalOutput")
        d_kT = nc.dram_tensor("d_kT", [P, (HDL // P) * T2], F32R,
                              kind="ExternalOutput")
        d_vho = nc.dram_tensor("d_vho", [P, (T2 // P) * (DK + 1) * 2 * (HDL // P)],
                               F32R, kind="ExternalOutput")
        d_at = nc.dram_tensor("d_at", [P, (T1 // 512) * (HDL // P) * 512], F32R,
                              kind="ExternalOutput")
        d_rc = nc.dram_tensor("d_rc", [P, (T1 // 512) * (HDL // P) * 1024], F32,
                              kind="ExternalOutput")

    with tile.TileContext(nc) as tc:
        with (
            tc.tile_pool(name="const", bufs=1) as pc,
            tc.tile_pool(name="big", bufs=1) as pb,
            tc.tile_pool(name="stream", bufs=2) as pst,
            tc.tile_pool(name="exp", bufs=2) as pe,
            tc.tile_pool(name="attn", bufs=n_pairs + 3) as pat,
            tc.tile_pool(name="small", bufs=2) as psm,
            tc.tile_pool(name="dscr", bufs=2, space="DRAM") as pdr,
            tc.tile_pool(name="psS", bufs=2, space="PSUM") as ppS,
            tc.tile_pool(name="psA", bufs=4, space="PSUM") as ppA,
        ):
            # ---- constants ----
            wq_s = pc.tile([P, n_kt, HDL], F32R, tag="wq")
            wk_s = pc.tile([P, n_kt, HDL], F32R, tag="wk")
            wv_s = pc.tile([P, n_kt, HDL], F32R, tag="wv")
            bq_s = pc.tile([P, n_pairs], F32, tag="bq")
            bk_s = pc.tile([P, n_pairs], F32, tag="bk")
            bv_s = pc.tile([DK, 2 * n_pairs], F32, tag="bv")
            nc.sync.dma_start(wq_s[:], wqT.rearrange("(ko p) m -> p ko m", p=P))
            nc.scalar.dma_start(wk_s[:], wkT.rearrange("(ko p) m -> p ko m", p=P))
            nc.sync.dma_start(wv_s[:], wvT.rearrange("(ko p) m -> p ko m", p=P))
            nc.scalar.dma_start(bq_s[:], bq[:])
            nc.scalar.dma_start(bk_s[:], bk[:])
            nc.scalar.dma_start(bv_s[:], bv[:])

            qT_s = pb.tile([P, n_pairs, T1], F32R, tag="qT")
            kT_s = pb.tile([P, n_pairs, T2], F32R, tag="kT")
            vho = pb.tile([P, n_t2t, (DK + 1) * 2 * n_pairs], F32R, tag="vho")
            nc.sync.dma_start(
                vho.rearrange("p t (h c) -> p t h c", c=DK + 1)[:, :, :, DK],
                ones[:, 0:n_t2t * 2 * n_pairs]
                .rearrange("p (t h) -> p t h", h=2 * n_pairs))

            # ---- phase A: projections ----
            # kT first (attention is gated on full kT), then v, then qT --
            # the first qT window unblocks attention while qT finishes.
            def v_chunk(t2m):
                ps = ppA.tile([P, 512], F32, tag="acc", name="vps")
                for k2 in range(n_kt // 4):
                    vt = pst.tile([P, 4, P], F32R, tag="vt", bufs=3)
                    eng = nc.sync if k2 % 2 == 0 else nc.scalar
                    eng.dma_start(
                        vt[:], vbT[4 * k2 * P:(4 * k2 + 4) * P,
                                   t2m * P:(t2m + 1) * P]
                        .rearrange("(f p) c -> p f c", p=P))
                    for kk in range(4):
                        k = 4 * k2 + kk
                        nc.tensor.matmul(ps[:, :HDL], vt[:, kk, :],
                                         wv_s[:, k, :],
                                         start=(k == 0), stop=(k == n_kt - 1))
                nc.vector.tensor_copy(
                    vho[:, t2m, :].rearrange("p (h c) -> p h c", c=DK + 1)
                    [:, :, 0:DK],
                    ps[:, :HDL].rearrange("p (h c) -> p h c", c=DK))

            def proj_qk(src, w_s, b_s, dst, v_per_t1b=0, v_base=0):
                Tn = src.shape[1]
                for t1b in range(Tn // 512):
                    if t1b % 2 == 0:
                        pss = [ppA.tile([P, 512], F32, tag="acc",
                                        name=f"ps{m}")
                               for m in range(n_pairs)]
                    else:
                        sa = ppS.tile([P, 1024], F32, tag="S", name="pjA")
                        sb = ppS.tile([P, 1024], F32, tag="S", name="pjB")
                        pss = [sa[:, 0:512], sa[:, 512:1024],
                               sb[:, 0:512], sb[:, 512:1024]]
                    for k4 in range(n_kt // 2):
                        st = pst.tile([P, 2, 512], F32R, tag="st", bufs=3)
                        eng = nc.sync if k4 % 2 == 0 else nc.scalar
                        eng.dma_start(
                            st[:], src[2 * k4 * P:(2 * k4 + 2) * P,
                                       t1b * 512:(t1b + 1) * 512]
                            .rearrange("(f p) c -> p f c", p=P))
                        for kk in range(2):
                            k = 2 * k4 + kk
                            for m in range(n_pairs):
                                nc.tensor.matmul(
                                    pss[m][:],
                                    w_s[:, k, m * P:(m + 1) * P],
                                    st[:, kk, :],
                                    start=(k == 0), stop=(k == n_kt - 1))
                    for m in range(n_pairs):
                        nc.vector.tensor_scalar_add(
                            dst[:, m, t1b * 512:(t1b + 1) * 512],
                            pss[m][:], b_s[:, m:m + 1])
                    for vi in range(v_per_t1b):
                        v_chunk(v_base + t1b * v_per_t1b + vi)

            # kT first with all v chunks interleaved (attention is gated on
            # kT + vho), then qT: its first window unblocks attention.
            proj_qk(kbT, wk_s, bk_s, kT_s, v_per_t1b=n_t2t // (T2 // 512))
            proj_qk(qbT, wq_s, bq_s, qT_s)

            # o-proj weights reuse wq's slot (phase A is done with it)
            wo_s = pc.tile([P, n_pairs, DM], F32R, tag="wq")
            nc.sync.dma_start(wo_s[:], woTs.rearrange("(ko p) n -> p ko n", p=P))

            if debug_outs:
                nc.sync.dma_start(
                    d_qT.rearrange("p (m t) -> p m t", m=n_pairs), qT_s[:])
                nc.sync.dma_start(
                    d_kT.rearrange("p (m t) -> p m t", m=n_pairs), kT_s[:])
                nc.sync.dma_start(
                    d_vho.rearrange("p (t c) -> p t c", t=n_t2t), vho[:])

            # ---- phase B: attention + o-proj per t1 window ----
            def emit_oproj(tiles, jj, t1b):
                ot = pat.tile([P, n_no * 512], F32, tag="ot", bufs=2)
                for n in range(n_no):
                    po = ppA.tile([P, 512], F32, tag="acc")
                    for p in range(n_pairs):
                        nc.tensor.matmul(
                            po[:],
                            tiles[p][:, t1b * P:(t1b + 1) * P],
                            wo_s[:, p, n * 512:(n + 1) * 512],
                            start=(p == 0), stop=(p == n_pairs - 1))
                    nc.vector.tensor_copy(
                        ot[:, n * 512:(n + 1) * 512], po[:])
                nc.sync.dma_start(
                    out[jj * 512 + t1b * P: jj * 512 + (t1b + 1) * P, :],
                    ot[:])

            prev_attn = prev_j = None
            for j in range(n_t1q):
                attn_tiles = []
                for p in range(n_pairs):
                    accA = ppA.tile([P, 512], F32, tag="acc")
                    accB = ppA.tile([P, 512], F32, tag="acc")
                    c0 = (DK + 1) * 2 * p

                    def emit_att(E, t2):
                        nc.tensor.matmul(
                            accA[0:DK + 1, :],
                            vho[:, t2, c0:c0 + DK + 1],
                            E[:, 0:512],
                            start=(t2 == 0), stop=(t2 == n_t2t - 1))
                        nc.tensor.matmul(
                            accB[0:DK + 1, :],
                            vho[:, t2, c0 + DK + 1:c0 + 2 * (DK + 1)],
                            E[:, 512:1024],
                            start=(t2 == 0), stop=(t2 == n_t2t - 1))

                    # software pipeline: att matmuls trail the exp by one
                    # t2 step so the in-order PE never waits on ScalarE.
                    prev = None
                    for t2 in range(n_t2t):
                        S = ppS.tile([P, 1024], F32, tag="S")
                        nc.tensor.matmul(
                            S[:, 0:512],
                            kT_s[0:DK, p, t2 * P:(t2 + 1) * P],
                            qT_s[0:DK, p, j * 512:(j + 1) * 512])
                        nc.tensor.matmul(
                            S[:, 512:1024],
                            kT_s[DK:P, p, t2 * P:(t2 + 1) * P],
                            qT_s[DK:P, p, j * 512:(j + 1) * 512])
                        E = pe.tile([P, 1024], F32R, tag="E")
                        nc.scalar.activation(E[:], S[:], EXP, scale=0.125)
                        if prev is not None:
                            emit_att(prev, t2 - 1)
                        prev = E
                    emit_att(prev, n_t2t - 1)
                    # normalize + bias, assemble attnT (128 x 512)
                    at = pat.tile([P, 512], F32R, tag="at")
                    rc = psm.tile([P, 1024], F32, tag="rc", bufs=1)
                    bc = psm.tile([DK, 1024], F32, tag="bc", bufs=1)
                    tmpB = psm.tile([DK, 512], F32R, tag="tmpB", bufs=1)
                    scr = pdr.tile([1, 1024], F32, tag="scr")
                    nc.vector.reciprocal(rc[DK:DK + 1, 0:512],
                                         accA[DK:DK + 1, :])
                    nc.vector.reciprocal(rc[DK:DK + 1, 512:1024],
                                         accB[DK:DK + 1, :])
                    nc.gpsimd.dma_start(scr[:], rc[DK:DK + 1, :])
                    nc.gpsimd.dma_start(bc[0:DK, :],
                                        scr.to_broadcast((DK, 1024)))
                    nc.vector.tensor_mul(at[0:DK, :], accA[0:DK, :],
                                         bc[0:DK, 0:512])
                    nc.vector.tensor_scalar_add(at[0:DK, :], at[0:DK, :],
                                                bv_s[:, 2 * p:2 * p + 1])
                    nc.vector.tensor_mul(tmpB[:], accB[0:DK, :],
                                         bc[0:DK, 512:1024])
                    nc.vector.tensor_scalar_add(tmpB[:], tmpB[:],
                                                bv_s[:, 2 * p + 1:2 * p + 2])
                    nc.gpsimd.dma_start(at[DK:P, :], tmpB[:])
                    attn_tiles.append(at)
                    # o-proj of the previous window, interleaved so the
                    # in-order PE never stalls ScalarE at window boundaries
                    if prev_attn is not None and p < 2:
                        emit_oproj(prev_attn, prev_j, 2 * p)
                        emit_oproj(prev_attn, prev_j, 2 * p + 1)
                    if debug_outs:
                        idx = (j * n_pairs + p)
                        nc.sync.dma_start(
                            d_at[:, idx * 512:(idx + 1) * 512], at[:])
                        nc.sync.dma_start(
                            d_rc[:, idx * 1024:(idx + 1) * 1024], rc[:])
                prev_attn, prev_j = attn_tiles, j
            for t1b in range(4):
                emit_oproj(prev_attn, prev_j, t1b)

    nc.compile()
    return nc


def make_in_maps(q, k, v, w_q, b_q, w_k, b_k, w_v, b_v, w_o, HDL=512):
    """Per-core host-side sharding/transposition. Core c: batch c//2, group c%2."""
    B = q.shape[0]
    n_pairs = HDL // P
    qT = [np.ascontiguousarray(q[b].T) for b in range(B)]
    kT = [np.ascontiguousarray(k[b].T) for b in range(B)]
    vT = [np.ascontiguousarray(v[b].T) for b in range(B)]
    in_maps = []
    for c in range(2 * B):
        b, g = c // 2, c % 2
        rows = slice(g * HDL, (g + 1) * HDL)
        in_maps.append({
            "qbT": qT[b], "kbT": kT[b], "vbT": vT[b],
            "wqT": np.ascontiguousarray(w_q[rows, :].T),
            "wkT": np.ascontiguousarray(w_k[rows, :].T),
            "wvT": np.ascontiguousarray(w_v[rows, :].T),
            "woTs": np.ascontiguousarray(w_o[:, rows].T),
            "ones": np.ones((P, 16 * 2 * n_pairs + DK), dtype=np.float32),
            "bq": np.ascontiguousarray(b_q[rows].reshape(n_pairs, P).T),
            "bk": np.ascontiguousarray(b_k[rows].reshape(n_pairs, P).T),
            "bv": np.ascontiguousarray(b_v[rows].reshape(2 * n_pairs, DK).T),
        })
    return in_maps


_NC = None


def kernel(q, k, v, mask, w_q, b_q, w_k, b_k, w_v, b_v, w_o, b_o):
    global _NC
    q = np.asarray(q, dtype=np.float32)
    k = np.asarray(k, dtype=np.float32)
    v = np.asarray(v, dtype=np.float32)
    w_q = np.asarray(w_q, dtype=np.float32)
    b_q = np.asarray(b_q, dtype=np.float32)
    w_k = np.asarray(w_k, dtype=np.float32)
    b_k = np.asarray(b_k, dtype=np.float32)
    w_v = np.asarray(w_v, dtype=np.float32)
    b_v = np.asarray(b_v, dtype=np.float32)
    w_o = np.asarray(w_o, dtype=np.float32)
    b_o = np.asarray(b_o, dtype=np.float32)
    # mask is all-ones by construction in this problem; unused.

    if _NC is None:
        _NC = build_nc()
    in_maps = make_in_maps(q, k, v, w_q, b_q, w_k, b_k, w_v, b_v, w_o)
    res = run_bass_kernel_spmd(_NC, in_maps, core_ids=list(range(8)))
    B, T1, DM = q.shape
    outp = np.empty((B, T1, DM), dtype=np.float32)
    for b in range(B):
        outp[b] = res.results[2 * b]["out"] + res.results[2 * b + 1]["out"] + b_o
    return outp



# revision 4
# speedup vs baseline: 1.2064x; 1.2064x over previous
"""Multi-head attention (B=4, T=2048, D=1024, H=16) on 8 NeuronCores.

Sharding: core c -> batch c//2, head-group c%2 (8 heads = 512 head-dims).
Host sums the two group partials per batch and adds b_o + b_v @ w_o.T
(b_v folds into the output bias; b_k cancels in softmax and is dropped).

All matmul operands bf16 (1 PE cycle/row regardless of free size).
Design (per core):
  projections: q/k/v chunks -> qT/kT [hd, t], vho [t2, h, 64+1(ones)]
  per unit (j-window, head-pair): 16x { scores [t2 128, t1 1024] -> PSUM,
    exp (ScalarE, scale=1/8) -> E bf16 }, flipped attn@V trailing by LAG:
    stationary E chunk [t2 128, t1c 128], moving vho [t2, 65] -> acc
    [t1 128, 65] PSUM (full PE utilization; ones col gives denominators).
  normalize: reciprocal of strided denoms + per-partition tensor_scalar mul
  XBAR DMA transpose (bf16) attn [t1, hd] -> attnT [hd, t1]
  o-proj: attnT stationary x w_o -> out.
Filler work (projections, o-proj) is deadline-scheduled into the exp-bound
unit steps to keep the PE busy; PSUM start=True zeroes a whole bank, so
only the first slice per bank starts an accumulation group.
"""

import heapq
import numpy as np
import ml_dtypes

import concourse.bacc as bacc
import concourse.mybir as mybir
import concourse.tile as tile
from concourse.bass_utils import run_bass_kernel_spmd

F32 = mybir.dt.float32
BF16 = mybir.dt.bfloat16
EXP = mybir.ActivationFunctionType.Exp

P = 128
DK = 64
DM = 1024
HDL = 512
T1 = 2048
T2 = 2048
NKT = DM // P        # 8 contraction tiles
NPAIR = HDL // P     # 4 head pairs
NT2T = T2 // P       # 16 t2 tiles
NJ = T1 // 512       # 4 t1 windows
NT1C = 4             # 128-wide t1 chunks per window
LAG = 4              # attn@V trails exp by LAG t2 steps


def build_nc(num_devices=8):
    nc = bacc.Bacc("TRN2", target_bir_lowering=False, debug=False,
                   num_devices=num_devices)

    qbT = nc.dram_tensor("qbT", [DM, T1], BF16, kind="ExternalInput")
    kbT = nc.dram_tensor("kbT", [DM, T2], BF16, kind="ExternalInput")
    vbT = nc.dram_tensor("vbT", [DM, T2], BF16, kind="ExternalInput")
    wqT = nc.dram_tensor("wqT", [DM, HDL], BF16, kind="ExternalInput")
    wkT = nc.dram_tensor("wkT", [DM, HDL], BF16, kind="ExternalInput")
    wvT = nc.dram_tensor("wvT", [DM, HDL], BF16, kind="ExternalInput")
    woTs = nc.dram_tensor("woTs", [HDL, DM], BF16, kind="ExternalInput")
    bq = nc.dram_tensor("bq", [P, NPAIR], F32, kind="ExternalInput")
    out = nc.dram_tensor("out", [T1, DM], F32, kind="ExternalOutput")

    with tile.TileContext(nc) as tc:
        with (
            tc.tile_pool(name="const", bufs=1) as pc,
            tc.tile_pool(name="big", bufs=1) as pb,
            tc.tile_pool(name="st", bufs=4) as pst,
            tc.tile_pool(name="E", bufs=11) as pe,
            tc.tile_pool(name="attn", bufs=8) as pat,
            tc.tile_pool(name="ot", bufs=2) as pot,
            tc.tile_pool(name="rc", bufs=2) as prc,
            tc.tile_pool(name="psS", bufs=2, space="PSUM") as ppS,
            tc.tile_pool(name="psW", bufs=2, space="PSUM") as ppW,
            tc.tile_pool(name="psA", bufs=2, space="PSUM") as ppA,
        ):
            # ---- constants / bulk inputs ----
            wq_s = pc.tile([P, NKT, HDL], BF16, tag="wq")
            wk_s = pc.tile([P, NKT, HDL], BF16, tag="wk")
            wv_s = pc.tile([P, NKT, HDL], BF16, tag="wv")
            wo_s = pc.tile([P, NPAIR, DM], BF16, tag="wo")
            bq_s = pc.tile([P, NPAIR], F32, tag="bq")
            vb_s = pb.tile([P, NKT, T2], BF16, tag="vb")
            qT_s = pb.tile([P, NPAIR, T1], BF16, tag="qT")
            kT_s = pb.tile([P, NPAIR, T2], BF16, tag="kT")
            vho = pb.tile([P, NT2T, 2 * NPAIR * 65], BF16, tag="vho")
            vho_r = vho.rearrange("p t (h c) -> p t h c", c=65)

            # SP queue: k/q path (order = arrival priority)
            nc.sync.dma_start(bq_s[:], bq[:])
            nc.sync.dma_start(wk_s[:], wkT.rearrange("(ko p) m -> p ko m", p=P))
            st_k = {}
            st_q = {}

            def issue_st(dst, src, idx):
                t = pst.tile([P, NKT, 512], BF16, tag="st")
                nc.sync.dma_start(
                    t[:], src.rearrange("(ko p) t -> p ko t", p=P)
                    [:, :, idx * 512:(idx + 1) * 512])
                dst[idx] = t

            issue_st(st_k, kbT, 0)
            nc.sync.dma_start(wq_s[:], wqT.rearrange("(ko p) m -> p ko m", p=P))
            issue_st(st_q, qbT, 0)

            # Pool queue: v path + wo; ones column via memset
            nc.gpsimd.memset(vho_r[:, :, :, 64], 1.0)
            nc.gpsimd.dma_start(
                vb_s[:, :, 0:512],
                vbT.rearrange("(ko p) t -> p ko t", p=P)[:, :, 0:512])
            nc.gpsimd.dma_start(wv_s[:], wvT.rearrange("(ko p) m -> p ko m", p=P))
            for c in range(1, 4):
                nc.gpsimd.dma_start(
                    vb_s[:, :, c * 512:(c + 1) * 512],
                    vbT.rearrange("(ko p) t -> p ko t", p=P)
                    [:, :, c * 512:(c + 1) * 512])
            nc.gpsimd.dma_start(wo_s[:], woTs.rearrange("(ko p) n -> p ko n", p=P))

            # ---- work items (PE filler), deadline-scheduled ----
            heap = []
            seq = [0]

            def push(due, fn):
                heapq.heappush(heap, (due, seq[0], fn))
                seq[0] += 1

            def kproj_chunk(t1b, pair, w_s, src_st, dst, bias):
                def fn():
                    ps = ppW.tile([P, 512], F32, tag="W")
                    st = src_st[t1b]
                    for kt in range(NKT):
                        nc.tensor.matmul(ps[:], w_s[:, kt, pair * P:(pair + 1) * P],
                                         st[:, kt, :],
                                         start=(kt == 0), stop=(kt == NKT - 1))
                    if bias is None:
                        nc.vector.tensor_copy(
                            dst[:, pair, t1b * 512:(t1b + 1) * 512], ps[:])
                    else:
                        nc.vector.tensor_scalar_add(
                            dst[:, pair, t1b * 512:(t1b + 1) * 512], ps[:],
                            bias[:, pair:pair + 1])
                return fn

            def vproj_chunk(t2t, pair):
                def fn():
                    ps = ppW.tile([P, 512], F32, tag="W")
                    for kt in range(NKT):
                        nc.tensor.matmul(
                            ps[:, 0:P],
                            vb_s[:, kt, t2t * P:(t2t + 1) * P],
                            wv_s[:, kt, pair * P:(pair + 1) * P],
                            start=(kt == 0), stop=(kt == NKT - 1))
                    nc.vector.tensor_copy(
                        vho_r[:, t2t, 2 * pair:2 * pair + 2, 0:64],
                        ps[:, 0:P].rearrange("p (h c) -> p h c", c=64))
                return fn

            # k-proj: t1b0 pairs 1-3 early (frees st slot); t1b>=1 per-pair JIT
            for pair in range(1, NPAIR):
                push(4 + pair, kproj_chunk(0, pair, wk_s, st_k, kT_s, None))
            for t1b in range(1, 4):
                push(4 * t1b - 8, lambda t1b=t1b: issue_st(st_k, kbT, t1b))
                for pair in range(NPAIR):
                    push(pair * 16 + 4 * t1b - 2,
                         kproj_chunk(t1b, pair, wk_s, st_k, kT_s, None))
            # v-proj
            for pair in range(NPAIR):
                for t2t in range(NT2T):
                    push(pair * 16 + t2t, vproj_chunk(t2t, pair))
            # q-proj: j0 pairs 1-3; j>=1 one unit early
            for pair in range(1, NPAIR):
                push(pair * 16 - 3, kproj_chunk(0, pair, wq_s, st_q, qT_s, bq_s))
            for j in range(1, NJ):
                push(j * 64 - 20, lambda j=j: issue_st(st_q, qbT, j))
                for pair in range(NPAIR):
                    push(j * 64 + pair * 16 - 8,
                         kproj_chunk(j, pair, wq_s, st_q, qT_s, bq_s))

            def drain(gstep):
                while heap and heap[0][0] <= gstep + 3:
                    heapq.heappop(heap)[2]()
                if heap:  # opportunistic: keep PE fed
                    heapq.heappop(heap)[2]()

            # ---- attention units ----
            attn_tiles = {}
            attnT_tiles = {}

            def oproj_chunk(j, t1c):
                def fn():
                    ot = pot.tile([P, DM], F32, tag="ot")
                    for n in range(2):
                        po = ppW.tile([P, 512], F32, tag="W")
                        for hdt in range(NPAIR):
                            nc.tensor.matmul(
                                po[:],
                                attnT_tiles[j][hdt][:, t1c * P:(t1c + 1) * P],
                                wo_s[:, hdt, n * 512:(n + 1) * 512],
                                start=(hdt == 0), stop=(hdt == NPAIR - 1))
                        nc.vector.tensor_copy(ot[:, n * 512:(n + 1) * 512], po[:])
                    nc.gpsimd.dma_start(
                        out[j * 512 + t1c * P:j * 512 + (t1c + 1) * P, :], ot[:])
                return fn

            def emit_attnv(jp, accs, Es, t2t):
                j, p = jp
                for t1c in range(NT1C):
                    i, half = t1c // 2, t1c % 2
                    for h in range(2):
                        nc.tensor.matmul(
                            accs[i][:, half * 130 + h * 65:
                                    half * 130 + (h + 1) * 65],
                            Es[t2t][:, h * 512 + t1c * P:h * 512 + (t1c + 1) * P],
                            vho_r[:, t2t, 2 * p + h, :],
                            start=(t2t == 0 and half == 0 and h == 0),
                            stop=(t2t == NT2T - 1),
                            skip_group_check=True)

            def emit_normalize(jp, accs):
                j, p = jp
                rc = prc.tile([P, 8], F32, tag="rc")
                for i in range(2):
                    nc.vector.reciprocal(
                        rc[:, 4 * i:4 * i + 4],
                        accs[i].rearrange("p (x c) -> p x c", c=65)[:, :, 64])
                for t1c in range(NT1C):
                    i, half = t1c // 2, t1c % 2
                    for h in range(2):
                        nc.vector.tensor_scalar_mul(
                            attn_tiles[j][t1c][:, p * P + h * 64:
                                               p * P + (h + 1) * 64],
                            accs[i][:, half * 130 + h * 65:
                                    half * 130 + h * 65 + 64],
                            rc[:, 4 * i + 2 * half + h:4 * i + 2 * half + h + 1])

            def window_done(j, gstep):
                # XBAR transpose attn [t1,hd] -> attnT [hd,t1], then o-proj
                attnT_tiles[j] = [pat.tile([P, 512], BF16, tag="atT",
                                           name=f"atT{j}_{t}")
                                  for t in range(NPAIR)]
                for hdt in range(NPAIR):
                    for t1c in range(NT1C):
                        nc.sync.dma_start(
                            attnT_tiles[j][hdt][:, t1c * P:(t1c + 1) * P],
                            attn_tiles[j][t1c][:, hdt * P:(hdt + 1) * P],
                            transpose=True)
                for t1c in range(NT1C):
                    push(gstep + 6 + t1c * 12, oproj_chunk(j, t1c))

            UNITS = [(j, p) for j in range(NJ) for p in range(NPAIR)]

            # prologue: minimal path to the first exp
            kproj_chunk(0, 0, wk_s, st_k, kT_s, None)()
            kproj_chunk(0, 0, wq_s, st_q, qT_s, bq_s)()

            prev = None  # (jp, accs, Es) of previous unit
            for u, jp in enumerate(UNITS):
                j, p = jp
                if p == 0:
                    attn_tiles[j] = [pat.tile([P, 512], BF16, tag="at",
                                              name=f"at{j}_{t}")
                                     for t in range(NT1C)]
                accs = [ppA.tile([P, 260], F32, tag="acc", name=f"a{u}_{i}")
                        for i in range(2)]
                Es = []
                for t2t in range(NT2T):
                    gstep = u * 16 + t2t
                    drain(gstep)
                    S = ppS.tile([P, 1024], F32, tag="S")
                    nc.tensor.matmul(S[:, 0:512],
                                     kT_s[0:DK, p, t2t * P:(t2t + 1) * P],
                                     qT_s[0:DK, p, j * 512:(j + 1) * 512])
                    nc.tensor.matmul(S[:, 512:1024],
                                     kT_s[DK:P, p, t2t * P:(t2t + 1) * P],
                                     qT_s[DK:P, p, j * 512:(j + 1) * 512])
                    E = pe.tile([P, 1024], BF16, tag="E")
                    nc.scalar.activation(E[:], S[:], EXP, scale=0.125)
                    Es.append(E)
                    if t2t >= LAG:
                        emit_attnv(jp, accs, Es, t2t - LAG)
                    elif prev is not None:
                        emit_attnv(prev[0], prev[1], prev[2],
                                   NT2T - LAG + t2t)
                        if t2t == LAG - 1:
                            emit_normalize(prev[0], prev[1])
                            if prev[0][1] == NPAIR - 1:
                                window_done(prev[0][0], u * 16)
                prev = (jp, accs, Es)

            # tail: drain last unit
            for r in range(LAG):
                emit_attnv(prev[0], prev[1], prev[2], NT2T - LAG + r)
            emit_normalize(prev[0], prev[1])
            window_done(prev[0][0], NJ * NPAIR * 16)
            while heap:
                heapq.heappop(heap)[2]()

    nc.compile()
    return nc


def make_in_maps(q, k, v, w_q, b_q, w_k, w_v, w_o):
    """Per-core host-side sharding. Core c: batch c//2, head-group c%2."""
    bf = ml_dtypes.bfloat16
    B = q.shape[0]
    qT = [np.ascontiguousarray(q[b].T).astype(bf) for b in range(B)]
    kT = [np.ascontiguousarray(k[b].T).astype(bf) for b in range(B)]
    vT = [np.ascontiguousarray(v[b].T).astype(bf) for b in range(B)]
    halves = []
    for g in range(2):
        rows = slice(g * HDL, (g + 1) * HDL)
        halves.append({
            "wqT": np.ascontiguousarray(w_q[rows, :].T).astype(bf),
            "wkT": np.ascontiguousarray(w_k[rows, :].T).astype(bf),
            "wvT": np.ascontiguousarray(w_v[rows, :].T).astype(bf),
            "woTs": np.ascontiguousarray(w_o[:, rows].T).astype(bf),
            "bq": np.ascontiguousarray(
                b_q[rows].reshape(NPAIR, P).T).astype(np.float32),
        })
    in_maps = []
    for c in range(2 * B):
        b, g = c // 2, c % 2
        m = {"qbT": qT[b], "kbT": kT[b], "vbT": vT[b]}
        m.update(halves[g])
        in_maps.append(m)
    return in_maps


_NC = None


def kernel(q, k, v, mask, w_q, b_q, w_k, b_k, w_v, b_v, w_o, b_o):
    global _NC
    q = np.asarray(q, dtype=np.float32)
    k = np.asarray(k, dtype=np.float32)
    v = np.asarray(v, dtype=np.float32)
    w_q = np.asarray(w_q, dtype=np.float32)
    b_q = np.asarray(b_q, dtype=np.float32)
    w_k = np.asarray(w_k, dtype=np.float32)
    w_v = np.asarray(w_v, dtype=np.float32)
    b_v = np.asarray(b_v, dtype=np.float32)
    w_o = np.asarray(w_o, dtype=np.float32)
    b_o = np.asarray(b_o, dtype=np.float32)
    # mask is all-ones by construction; b_k cancels in softmax; b_v folds
    # into the output bias.

    if _NC is None:
        _NC = build_nc()
    in_maps = make_in_maps(q, k, v, w_q, b_q, w_k, w_v, w_o)
    res = run_bass_kernel_spmd(_NC, in_maps, core_ids=list(range(8)))
    b_eff = (b_o + b_v @ w_o.T).astype(np.float32)
    B = q.shape[0]
    outp = np.empty((B, T1, DM), dtype=np.float32)
    for b in range(B):
        outp[b] = res.results[2 * b]["out"] + res.results[2 * b + 1]["out"] + b_eff
    return outp
